# revision 7
# baseline (speedup 1.0000x reference)
"""Depth-aware 3x3 convolution on 8 Trainium2 NeuronCores (Bass, raw engine blocks).

out[b,o,h,w] = sum_{c,kh,kw} weight[o,c,kh,kw] * x[b,c,h+kh-1,w+kw-1]
                             * exp(-8.3*|depth[b,h,w] - depth[b,h+kh-1,w+kw-1]|)

Sharding: core = 2*b + (h >= 128); each core computes a [32, 128, 256] output
slab from a 130-row padded input frame (1-row halo from the host slice).

Datapath is bf16 (x, weight, sim, modulated product, output) with f32 depth
and f32 PSUM accumulation; the DVE modulation multiply runs in 2x perf mode
(all operands contiguous, 4B-aligned, pitch-256 pre-shifted on the host).

Per-core pipeline (ring-4 buffering everywhere to hide ~2us DMA completion
latency; one DMA per logical transfer):
  A. sim: depth rows pixel-major [128, 258]x3 -> sub (DVE) -> |.| (DVE STT)
     -> exp (ACT, bf16) -> one DMA -> DRAM simd[9, 32768]
  B. main loop over 16 tiles of 2048 px (8 rows):
     - DMA: x3 chunk [96, 10*256] bf16 (3 column-shift blocks on partitions)
     - DMA: one broadcast per pass: simd[3t:3t+3] -> simrep3 [96, 2048] bf16
     - DVE: xm3 = x3[:, t*256 : t*256+2048] * simrep3  (bf16 2x)  t=0,1,2
     - PE : psum[32, 2048] += w3[:, t].T @ xm3  (K=96, N=512 x4, bf16)
     - ACT: psum -> out_sb bf16; DMA out.

The program body sits in a per-engine hardware loop (`trips`) with a
two-phase leader-follower barrier and exact semaphore reset (sem_clear)
between trips, so one NEFF re-executes the kernel N times; device time is
measured as the wall-clock slope between two trip counts.  Grading uses
trips=1.
"""
import sys

import numpy as np

sys.path.insert(0, "/opt/trn_rl_repo")

import concourse.bass as bass
import concourse.mybir as mybir
from concourse.bass_utils import run_bass_kernel_spmd

F32 = mybir.dt.float32
BF16 = mybir.dt.bfloat16
EXP = mybir.ActivationFunctionType.Exp

B, C, H, W = 4, 32, 256, 256
O = 32
ALPHA = 8.3
R = 128  # output rows per core
WP = W + 2  # padded width (depth frame only)
FR = R + 2  # frame rows per core
NPIX = R * W  # 32768
TROWS = 8  # rows per tile
TILE = TROWS * W  # 2048
NT = R // TROWS  # 16
CH_ROWS = TROWS + 2  # x3 chunk rows
CH = CH_ROWS * W  # x3 chunk elems per partition
MMN = 512  # matmul free-dim chunk
QN = TILE // MMN  # 4
NB = 4  # ring depth


def build_nc(trips=1):
    nc = bass.Bass("TRN2", target_bir_lowering=False, debug=False, num_devices=8)
    # x3: 3 column-shift blocks stacked on partitions, pitch-256 rows
    x3_in = nc.declare_dram_parameter("x3", [96, FR * W], BF16, isOutput=False)
    dp_in = nc.declare_dram_parameter("dp", [FR, WP], F32, isOutput=False)
    w3_in = nc.declare_dram_parameter("w3", [96, 96], BF16, isOutput=False)
    out_d = nc.declare_dram_parameter("out", [O, NPIX], BF16, isOutput=True)
    simd = nc.dram_tensor("simd", [9, NPIX], BF16)

    from contextlib import ExitStack

    ctx = ExitStack()
    with ctx:
        d_sb = ctx.enter_context(nc.sbuf_tensor([128, 3 * WP], F32))
        adiff9 = ctx.enter_context(nc.sbuf_tensor([128, 9 * W], F32))
        sim9 = ctx.enter_context(nc.sbuf_tensor([128, 9 * W], BF16))
        w3_sb = ctx.enter_context(nc.sbuf_tensor([96, 96], BF16))
        x3c = ctx.enter_context(nc.sbuf_tensor([96, NB * CH], BF16))
        simrep3 = ctx.enter_context(nc.sbuf_tensor([96, NB * TILE], BF16))
        xm3 = ctx.enter_context(nc.sbuf_tensor([96, NB * TILE], BF16))
        out_sb = ctx.enter_context(nc.sbuf_tensor([32, NB * TILE], BF16))
        psum = ctx.enter_context(nc.psum_tensor([32, 2 * TILE], F32))
        ld_sem = ctx.enter_context(nc.semaphore("ld_sem"))
        sim_dve = ctx.enter_context(nc.semaphore("sim_dve"))
        act_exp = ctx.enter_context(nc.semaphore("act_exp"))
        sim_st = ctx.enter_context(nc.semaphore("sim_st"))
        x_q = [ctx.enter_context(nc.semaphore(f"x_q{r}")) for r in range(NB)]
        bc_q = [ctx.enter_context(nc.semaphore(f"bc_q{r}")) for r in range(NB)]
        st_q = [ctx.enter_context(nc.semaphore(f"st_q{r}")) for r in range(NB)]
        mod_sem = ctx.enter_context(nc.semaphore("mod_sem"))
        pe_sem = ctx.enter_context(nc.semaphore("pe_sem"))
        act_cp = ctx.enter_context(nc.semaphore("act_cp"))
        bar_g = ctx.enter_context(nc.semaphore("bar_g"))
        bar_r = ctx.enter_context(nc.semaphore("bar_r"))
        bar_a = ctx.enter_context(nc.semaphore("bar_a"))
        bar_r2 = ctx.enter_context(nc.semaphore("bar_r2"))
        block = ctx.enter_context(nc.Block())

        PIPE_SEMS = (
            [ld_sem, sim_dve, act_exp, sim_st, mod_sem, pe_sem, act_cp]
            + x_q + bc_q + st_q
        )

        def follower_barrier(eng):
            # two-phase: park on bar_r while SP resets pipe sems, then ack and
            # park on bar_r2 while SP resets bar_r.  All wait values are
            # trip-invariant; every sem returns to 0 each trip.
            eng.drain()
            eng.sem_inc(bar_g, 1)
            eng.wait_ge(bar_r, 1)
            eng.sem_inc(bar_a, 1)
            eng.wait_ge(bar_r2, 1)

        # broadcast source view: [3, 32(bcast), 2048] per (i, t)
        simd_b = simd.ap().rearrange("k (u n) -> k u n", u=1)

        @block.sync
        def _(sync: bass.BassEngine):
            with sync.Fori(0, trips):
                # startup loads: d (3 row-shifted views), w3
                for t in range(3):
                    sync.dma_start(
                        d_sb[:, t * WP : (t + 1) * WP], dp_in[t : t + 128, :]
                    ).then_inc(ld_sem, 16)
                sync.dma_start(w3_sb[:], w3_in[:]).then_inc(ld_sem, 16)
                # sim -> DRAM in one DMA: simd[k, r*W+w] <- sim9[r, k*W+w]
                sync.wait_ge(act_exp, 9)
                sync.dma_start(
                    simd.ap().rearrange("k (r w) -> r k w", r=R),
                    sim9.ap().rearrange("r (k w) -> r k w", k=9),
                ).then_inc(sim_st, 16)
                # main loop
                for i in range(NT):
                    bi = i % NB
                    # x3 chunk for tile i
                    if i >= NB:
                        sync.wait_ge(mod_sem, 3 * (i - NB) + 3)
                    sync.dma_start(
                        x3c[:, bi * CH : (bi + 1) * CH],
                        x3_in[:, i * TROWS * W : i * TROWS * W + CH],
                    ).then_inc(x_q[bi], 16)
                    # one broadcast DMA per pass
                    if i == 0:
                        sync.wait_ge(sim_st, 16)
                    for t in range(3):
                        s = 3 * i + t
                        sb = s % NB
                        if s >= NB:
                            sync.wait_ge(mod_sem, s - NB + 1)
                        sync.dma_start(
                            simrep3[:, sb * TILE : (sb + 1) * TILE],
                            simd_b[
                                3 * t : 3 * t + 3, :, i * TILE : (i + 1) * TILE
                            ].to_broadcast((3, 32, TILE)),
                        ).then_inc(bc_q[sb], 16)
                    # store tile i-1
                    if i >= 1:
                        sync.wait_ge(act_cp, i)
                        sync.dma_start(
                            out_d[:, (i - 1) * TILE : i * TILE],
                            out_sb[:, ((i - 1) % NB) * TILE : ((i - 1) % NB + 1) * TILE],
                        ).then_inc(st_q[(i - 1) % NB], 16)
                sync.wait_ge(act_cp, NT)
                sync.dma_start(
                    out_d[:, (NT - 1) * TILE :],
                    out_sb[:, ((NT - 1) % NB) * TILE : ((NT - 1) % NB + 1) * TILE],
                ).then_inc(st_q[(NT - 1) % NB], 16)
                # ---- trip barrier: leader ----
                # all DMA completions at their exact per-trip finals
                sync.wait_ge(ld_sem, 64)
                sync.wait_ge(sim_st, 16)
                for r in range(NB):
                    sync.wait_ge(x_q[r], 64)
                    sync.wait_ge(bc_q[r], 192)
                    sync.wait_ge(st_q[r], 64)
                # phase 1: engines idle (parked on bar_r); reset pipe sems
                sync.wait_ge(bar_g, 3)
                for sem in PIPE_SEMS:
                    sync.sem_clear(sem)
                sync.sem_clear(bar_g)
                sync.sem_clear(bar_r2)
                sync.sem_inc(bar_r, 1)
                # phase 2: engines parked on bar_r2; reset bar_r and release
                sync.wait_ge(bar_a, 3)
                sync.sem_clear(bar_r)
                sync.sem_clear(bar_a)
                sync.sem_inc(bar_r2, 1)

        @block.vector
        def _(vector):
            with vector.Fori(0, trips):
                # sim phase: diff + abs per tap
                vector.wait_ge(ld_sem, 64)
                for t in range(3):
                    for j in range(3):
                        k = 3 * t + j
                        vector.tensor_sub(
                            adiff9[:, k * W : (k + 1) * W],
                            d_sb[:, WP + 1 : WP + 1 + W],
                            d_sb[:, t * WP + j : t * WP + j + W],
                        )
                        vector.drain()
                        vector.scalar_tensor_tensor(
                            adiff9[:, k * W : (k + 1) * W],
                            adiff9[:, k * W : (k + 1) * W],
                            -1.0,
                            adiff9[:, k * W : (k + 1) * W],
                            op0=mybir.AluOpType.mult,
                            op1=mybir.AluOpType.max,
                        ).then_inc(sim_dve, 1)
                # modulation loop
                for i in range(NT):
                    bi = i % NB
                    vector.wait_ge(x_q[bi], 16 * (i // NB + 1))
                    for t in range(3):
                        s = 3 * i + t
                        sb = s % NB
                        vector.wait_ge(bc_q[sb], 16 * (s // NB + 1))
                        if s >= NB:
                            vector.wait_ge(pe_sem, s - NB + 1)
                        vector.tensor_mul(
                            xm3[:, sb * TILE : (sb + 1) * TILE],
                            x3c[:, bi * CH + t * W : bi * CH + t * W + TILE],
                            simrep3[:, sb * TILE : (sb + 1) * TILE],
                        ).then_inc(mod_sem, 1)
                follower_barrier(vector)

        @block.tensor
        def _(tensor):
            with tensor.Fori(0, trips):
                tensor.wait_ge(ld_sem, 64)
                for i in range(NT):
                    pb = i % 2
                    if i >= 2:
                        tensor.wait_ge(act_cp, i - 1)
                    for t in range(3):
                        s = 3 * i + t
                        sb = s % NB
                        tensor.wait_ge(mod_sem, s + 1)
                        for q in range(QN):
                            mm = tensor.matmul(
                                psum[
                                    :, pb * TILE + q * MMN : pb * TILE + (q + 1) * MMN
                                ],
                                w3_sb[:, 32 * t : 32 * (t + 1)],
                                xm3[:, sb * TILE + q * MMN : sb * TILE + (q + 1) * MMN],
                                start=(t == 0),
                                stop=(t == 2),
                            )
                            if q == QN - 1:
                                mm.then_inc(pe_sem, 1)
                follower_barrier(tensor)

        @block.scalar
        def _(scalar):
            with scalar.Fori(0, trips):
                # exp per tap (bf16 out)
                for k in range(9):
                    scalar.wait_ge(sim_dve, k + 1)
                    scalar.activation(
                        sim9[:, k * W : (k + 1) * W],
                        adiff9[:, k * W : (k + 1) * W],
                        EXP,
                        scale=-ALPHA,
                    ).then_inc(act_exp, 1)
                # psum -> sbuf copies
                for i in range(NT):
                    pb = i % 2
                    ob = i % NB
                    scalar.wait_ge(pe_sem, 3 * i + 3)
                    if i >= NB:
                        scalar.wait_ge(st_q[ob], 16 * (i // NB))
                    scalar.copy(
                        out_sb[:, ob * TILE : (ob + 1) * TILE],
                        psum[:, pb * TILE : (pb + 1) * TILE],
                    ).then_inc(act_cp, 1)
                follower_barrier(scalar)

    return nc


_NC_CACHE = {}


def _get_nc(trips=1):
    if trips not in _NC_CACHE:
        _NC_CACHE[trips] = build_nc(trips)
    return _NC_CACHE[trips]


def _prep_core(x, depth, core):
    import ml_dtypes

    b, half = core // 2, core % 2
    r0 = half * R
    # padded frame [C, FR, WP]: image rows r0-1 .. r0+R, zero-padded
    xpad = np.zeros((C, FR, WP), dtype=np.float32)
    dpad = np.zeros((FR, WP), dtype=np.float32)
    lo, hi = r0 - 1, r0 + R + 1
    slo, shi = max(lo, 0), min(hi, H)
    xpad[:, slo - lo : shi - lo, 1 : 1 + W] = x[b, :, slo:shi, :]
    dpad[slo - lo : shi - lo, 1 : 1 + W] = depth[b, 0, slo:shi, :]
    # x3: 3 column-shift blocks stacked on partitions, pitch-256 (pre-shifted)
    x3 = np.empty((3, C, FR, W), dtype=np.float32)
    x3[0] = xpad[:, :, 0:W]  # j=0: w-1
    x3[1] = xpad[:, :, 1 : 1 + W]  # j=1: w
    x3[2] = xpad[:, :, 2 : 2 + W]  # j=2: w+1
    return {
        "x3": x3.reshape(3 * C, FR * W).astype(ml_dtypes.bfloat16),
        "dp": dpad,
        "w3": None,  # filled by caller (shared)
    }


def _prep_inputs(x, depth, weight):
    import ml_dtypes

    x = np.ascontiguousarray(x, dtype=np.float32)
    depth = np.ascontiguousarray(depth, dtype=np.float32)
    weight = np.ascontiguousarray(weight, dtype=np.float32)
    # w3[32j + c, 32t + o] = weight[o, c, t, j]
    w3 = (
        np.transpose(weight, (3, 1, 2, 0))
        .reshape(96, 96)
        .astype(ml_dtypes.bfloat16)
        .copy()
    )
    in_maps = []
    for core in range(8):
        m = _prep_core(x, depth, core)
        m["w3"] = w3
        in_maps.append(m)
    return in_maps


def kernel(x, depth, weight):
    in_maps = _prep_inputs(x, depth, weight)
    nc = _get_nc(1)
    res = run_bass_kernel_spmd(nc, in_maps, list(range(8)))

    out = np.empty((B, O, H, W), dtype=np.float32)
    for core in range(8):
        b, half = core // 2, core % 2
        out[b, :, half * R : (half + 1) * R, :] = (
            res.results[core]["out"].astype(np.float32).reshape(O, R, W)
        )
    return out


# revision 9
# speedup vs baseline: 1.0211x; 1.0211x over previous
"""Depth-aware 3x3 convolution on 8 Trainium2 NeuronCores (Bass, raw engine blocks).

out[b,o,h,w] = sum_{c,kh,kw} weight[o,c,kh,kw] * x[b,c,h+kh-1,w+kw-1]
                             * exp(-8.3*|depth[b,h,w] - depth[b,h+kh-1,w+kw-1]|)

Sharding: core = 2*b + (h >= 128); each core computes a [32, 128, 256] output
slab from a 130-row padded input frame (1-row halo from the host slice).

Datapath is bf16 (x, weight, sim, modulated product, output) with f32 depth
and f32 PSUM accumulation; the DVE modulation multiply runs in 2x perf mode
(all operands contiguous, 4B-aligned, pitch-256 pre-shifted on the host).

Per-core pipeline (ring-4 buffering everywhere to hide ~2us DMA completion
latency; one DMA per logical transfer):
  A. sim: depth rows pixel-major [128, 258]x3 -> sub (DVE) -> |.| (DVE STT)
     -> exp (ACT, bf16) -> one DMA -> DRAM simd[9, 32768]
  B. main loop over 16 tiles of 2048 px (8 rows):
     - DMA: x3 chunk [96, 10*256] bf16 (3 column-shift blocks on partitions)
     - DMA: one broadcast per pass: simd[3t:3t+3] -> simrep3 [96, 2048] bf16
     - DVE: xm3 = x3[:, t*256 : t*256+2048] * simrep3  (bf16 2x)  t=0,1,2
     - PE : psum[32, 2048] += w3[:, t].T @ xm3  (K=96, N=512 x4, bf16)
     - ACT: psum -> out_sb bf16; DMA out.

The program body sits in a per-engine hardware loop (`trips`) with a
two-phase leader-follower barrier and exact semaphore reset (sem_clear)
between trips, so one NEFF re-executes the kernel N times; device time is
measured as the wall-clock slope between two trip counts.  Grading uses
trips=1.
"""
import sys

import numpy as np

sys.path.insert(0, "/opt/trn_rl_repo")

import concourse.bass as bass
import concourse.mybir as mybir
from concourse.bass_utils import run_bass_kernel_spmd

F32 = mybir.dt.float32
BF16 = mybir.dt.bfloat16
EXP = mybir.ActivationFunctionType.Exp

B, C, H, W = 4, 32, 256, 256
O = 32
ALPHA = 8.3
R = 128  # output rows per core
WP = W + 2  # padded width (depth frame only)
FR = R + 2  # frame rows per core
NPIX = R * W  # 32768
TROWS = 8  # rows per tile
TILE = TROWS * W  # 2048
NT = R // TROWS  # 16
CH_ROWS = TROWS + 2  # x3 chunk rows
CH = CH_ROWS * W  # x3 chunk elems per partition
MMN = 512  # matmul free-dim chunk
QN = TILE // MMN  # 4
NB = 4  # ring depth


def build_nc(trips=1):
    nc = bass.Bass("TRN2", target_bir_lowering=False, debug=False, num_devices=8)
    # x3: 3 column-shift blocks stacked on partitions, pitch-256 rows
    x3_in = nc.declare_dram_parameter("x3", [96, FR * W], BF16, isOutput=False)
    dp_in = nc.declare_dram_parameter("dp", [FR, WP], F32, isOutput=False)
    w3_in = nc.declare_dram_parameter("w3", [96, 96], BF16, isOutput=False)
    out_d = nc.declare_dram_parameter("out", [O, NPIX], BF16, isOutput=True)
    simd = nc.dram_tensor("simd", [9, NPIX], BF16)

    from contextlib import ExitStack

    ctx = ExitStack()
    with ctx:
        d_sb = ctx.enter_context(nc.sbuf_tensor([128, 3 * WP], F32))
        adiff9 = ctx.enter_context(nc.sbuf_tensor([128, 9 * W], F32))
        sim9 = ctx.enter_context(nc.sbuf_tensor([128, 9 * W], BF16))
        w3_sb = ctx.enter_context(nc.sbuf_tensor([96, 96], BF16))
        x3c = ctx.enter_context(nc.sbuf_tensor([96, NB * CH], BF16))
        simrep3 = ctx.enter_context(nc.sbuf_tensor([96, NB * TILE], BF16))
        xm3 = ctx.enter_context(nc.sbuf_tensor([96, NB * TILE], BF16))
        out_sb = ctx.enter_context(nc.sbuf_tensor([32, NB * TILE], BF16))
        psum = ctx.enter_context(nc.psum_tensor([32, 2 * TILE], F32))
        ld_sem = ctx.enter_context(nc.semaphore("ld_sem"))
        sim_dve = ctx.enter_context(nc.semaphore("sim_dve"))
        act_exp = ctx.enter_context(nc.semaphore("act_exp"))
        sim_st = ctx.enter_context(nc.semaphore("sim_st"))
        x_q = [ctx.enter_context(nc.semaphore(f"x_q{r}")) for r in range(NB)]
        bc_q = [ctx.enter_context(nc.semaphore(f"bc_q{r}")) for r in range(NB)]
        st_q = [ctx.enter_context(nc.semaphore(f"st_q{r}")) for r in range(NB)]
        mod_sem = ctx.enter_context(nc.semaphore("mod_sem"))
        pe_sem = ctx.enter_context(nc.semaphore("pe_sem"))
        act_cp = ctx.enter_context(nc.semaphore("act_cp"))
        bar_g = ctx.enter_context(nc.semaphore("bar_g"))
        bar_r = ctx.enter_context(nc.semaphore("bar_r"))
        bar_a = ctx.enter_context(nc.semaphore("bar_a"))
        bar_r2 = ctx.enter_context(nc.semaphore("bar_r2"))
        block = ctx.enter_context(nc.Block())

        PIPE_SEMS = (
            [ld_sem, sim_dve, act_exp, sim_st, mod_sem, pe_sem, act_cp]
            + x_q + bc_q + st_q
        )

        def follower_barrier(eng):
            # two-phase: park on bar_r while SP resets pipe sems, then ack and
            # park on bar_r2 while SP resets bar_r.  All wait values are
            # trip-invariant; every sem returns to 0 each trip.
            eng.drain()
            eng.sem_inc(bar_g, 1)
            eng.wait_ge(bar_r, 1)
            eng.sem_inc(bar_a, 1)
            eng.wait_ge(bar_r2, 1)

        simd_r = simd.ap().rearrange("k (r w) -> k r w", r=R)

        @block.sync
        def _(sync: bass.BassEngine):
            with sync.Fori(0, trips):
                # startup loads: d (3 row-shifted views), w3
                for t in range(3):
                    sync.dma_start(
                        d_sb[:, t * WP : (t + 1) * WP], dp_in[t : t + 128, :]
                    ).then_inc(ld_sem, 16)
                sync.dma_start(w3_sb[:], w3_in[:]).then_inc(ld_sem, 16)
                # sim -> DRAM
                for k in range(9):
                    sync.wait_ge(act_exp, k + 1)
                    sync.dma_start(
                        simd_r[k], sim9[:, k * W : (k + 1) * W]
                    ).then_inc(sim_st, 16)
                # main loop
                for i in range(NT):
                    bi = i % NB
                    # x3 chunk for tile i
                    if i >= NB:
                        sync.wait_ge(mod_sem, 3 * (i - NB) + 3)
                    sync.dma_start(
                        x3c[:, bi * CH : (bi + 1) * CH],
                        x3_in[:, i * TROWS * W : i * TROWS * W + CH],
                    ).then_inc(x_q[bi], 16)
                    # one broadcast DMA per pass
                    if i == 0:
                        sync.wait_ge(sim_st, 144)
                    for t in range(3):
                        s = 3 * i + t
                        sb = s % NB
                        if s >= NB:
                            sync.wait_ge(mod_sem, s - NB + 1)
                        for j in range(3):
                            sync.dma_start(
                                simrep3[
                                    32 * j : 32 * (j + 1),
                                    sb * TILE : (sb + 1) * TILE,
                                ],
                                simd[
                                    3 * t + j : 3 * t + j + 1,
                                    i * TILE : (i + 1) * TILE,
                                ].to_broadcast((32, TILE)),
                            ).then_inc(bc_q[sb], 16)
                    # store tile i-1
                    if i >= 1:
                        sync.wait_ge(act_cp, i)
                        sync.dma_start(
                            out_d[:, (i - 1) * TILE : i * TILE],
                            out_sb[:, ((i - 1) % NB) * TILE : ((i - 1) % NB + 1) * TILE],
                        ).then_inc(st_q[(i - 1) % NB], 16)
                sync.wait_ge(act_cp, NT)
                sync.dma_start(
                    out_d[:, (NT - 1) * TILE :],
                    out_sb[:, ((NT - 1) % NB) * TILE : ((NT - 1) % NB + 1) * TILE],
                ).then_inc(st_q[(NT - 1) % NB], 16)
                # ---- trip barrier: leader ----
                # all DMA completions at their exact per-trip finals
                sync.wait_ge(ld_sem, 64)
                sync.wait_ge(sim_st, 144)
                for r in range(NB):
                    sync.wait_ge(x_q[r], 64)
                    sync.wait_ge(bc_q[r], 576)
                    sync.wait_ge(st_q[r], 64)
                # phase 1: engines idle (parked on bar_r); reset pipe sems
                sync.wait_ge(bar_g, 3)
                for sem in PIPE_SEMS:
                    sync.sem_clear(sem)
                sync.sem_clear(bar_g)
                sync.sem_clear(bar_r2)
                sync.sem_inc(bar_r, 1)
                # phase 2: engines parked on bar_r2; reset bar_r and release
                sync.wait_ge(bar_a, 3)
                sync.sem_clear(bar_r)
                sync.sem_clear(bar_a)
                sync.sem_inc(bar_r2, 1)

        @block.vector
        def _(vector):
            with vector.Fori(0, trips):
                # sim phase: diff + abs per tap
                vector.wait_ge(ld_sem, 64)
                for t in range(3):
                    for j in range(3):
                        k = 3 * t + j
                        vector.tensor_sub(
                            adiff9[:, k * W : (k + 1) * W],
                            d_sb[:, WP + 1 : WP + 1 + W],
                            d_sb[:, t * WP + j : t * WP + j + W],
                        )
                        vector.drain()
                        vector.scalar_tensor_tensor(
                            adiff9[:, k * W : (k + 1) * W],
                            adiff9[:, k * W : (k + 1) * W],
                            -1.0,
                            adiff9[:, k * W : (k + 1) * W],
                            op0=mybir.AluOpType.mult,
                            op1=mybir.AluOpType.max,
                        ).then_inc(sim_dve, 1)
                # modulation loop
                for i in range(NT):
                    bi = i % NB
                    vector.wait_ge(x_q[bi], 16 * (i // NB + 1))
                    for t in range(3):
                        s = 3 * i + t
                        sb = s % NB
                        vector.wait_ge(bc_q[sb], 48 * (s // NB + 1))
                        if s >= NB:
                            vector.wait_ge(pe_sem, s - NB + 1)
                        vector.tensor_mul(
                            xm3[:, sb * TILE : (sb + 1) * TILE],
                            x3c[:, bi * CH + t * W : bi * CH + t * W + TILE],
                            simrep3[:, sb * TILE : (sb + 1) * TILE],
                        ).then_inc(mod_sem, 1)
                follower_barrier(vector)

        @block.tensor
        def _(tensor):
            with tensor.Fori(0, trips):
                tensor.wait_ge(ld_sem, 64)
                for i in range(NT):
                    pb = i % 2
                    if i >= 2:
                        tensor.wait_ge(act_cp, i - 1)
                    for t in range(3):
                        s = 3 * i + t
                        sb = s % NB
                        tensor.wait_ge(mod_sem, s + 1)
                        for q in range(QN):
                            mm = tensor.matmul(
                                psum[
                                    :, pb * TILE + q * MMN : pb * TILE + (q + 1) * MMN
                                ],
                                w3_sb[:, 32 * t : 32 * (t + 1)],
                                xm3[:, sb * TILE + q * MMN : sb * TILE + (q + 1) * MMN],
                                start=(t == 0),
                                stop=(t == 2),
                            )
                            if q == QN - 1:
                                mm.then_inc(pe_sem, 1)
                follower_barrier(tensor)

        @block.scalar
        def _(scalar):
            with scalar.Fori(0, trips):
                # exp per tap (bf16 out)
                for k in range(9):
                    scalar.wait_ge(sim_dve, k + 1)
                    scalar.activation(
                        sim9[:, k * W : (k + 1) * W],
                        adiff9[:, k * W : (k + 1) * W],
                        EXP,
                        scale=-ALPHA,
                    ).then_inc(act_exp, 1)
                # psum -> sbuf copies
                for i in range(NT):
                    pb = i % 2
                    ob = i % NB
                    scalar.wait_ge(pe_sem, 3 * i + 3)
                    if i >= NB:
                        scalar.wait_ge(st_q[ob], 16 * (i // NB))
                    scalar.copy(
                        out_sb[:, ob * TILE : (ob + 1) * TILE],
                        psum[:, pb * TILE : (pb + 1) * TILE],
                    ).then_inc(act_cp, 1)
                follower_barrier(scalar)

    return nc


_NC_CACHE = {}


def _get_nc(trips=1):
    if trips not in _NC_CACHE:
        _NC_CACHE[trips] = build_nc(trips)
    return _NC_CACHE[trips]


def _prep_core(x, depth, core):
    import ml_dtypes

    b, half = core // 2, core % 2
    r0 = half * R
    # padded frame [C, FR, WP]: image rows r0-1 .. r0+R, zero-padded
    xpad = np.zeros((C, FR, WP), dtype=np.float32)
    dpad = np.zeros((FR, WP), dtype=np.float32)
    lo, hi = r0 - 1, r0 + R + 1
    slo, shi = max(lo, 0), min(hi, H)
    xpad[:, slo - lo : shi - lo, 1 : 1 + W] = x[b, :, slo:shi, :]
    dpad[slo - lo : shi - lo, 1 : 1 + W] = depth[b, 0, slo:shi, :]
    # x3: 3 column-shift blocks stacked on partitions, pitch-256 (pre-shifted)
    x3 = np.empty((3, C, FR, W), dtype=np.float32)
    x3[0] = xpad[:, :, 0:W]  # j=0: w-1
    x3[1] = xpad[:, :, 1 : 1 + W]  # j=1: w
    x3[2] = xpad[:, :, 2 : 2 + W]  # j=2: w+1
    return {
        "x3": x3.reshape(3 * C, FR * W).astype(ml_dtypes.bfloat16),
        "dp": dpad,
        "w3": None,  # filled by caller (shared)
    }


def _prep_inputs(x, depth, weight):
    import ml_dtypes

    x = np.ascontiguousarray(x, dtype=np.float32)
    depth = np.ascontiguousarray(depth, dtype=np.float32)
    weight = np.ascontiguousarray(weight, dtype=np.float32)
    # w3[32j + c, 32t + o] = weight[o, c, t, j]
    w3 = (
        np.transpose(weight, (3, 1, 2, 0))
        .reshape(96, 96)
        .astype(ml_dtypes.bfloat16)
        .copy()
    )
    in_maps = []
    for core in range(8):
        m = _prep_core(x, depth, core)
        m["w3"] = w3
        in_maps.append(m)
    return in_maps


def kernel(x, depth, weight):
    in_maps = _prep_inputs(x, depth, weight)
    nc = _get_nc(1)
    res = run_bass_kernel_spmd(nc, in_maps, list(range(8)))

    out = np.empty((B, O, H, W), dtype=np.float32)
    for core in range(8):
        b, half = core // 2, core % 2
        out[b, :, half * R : (half + 1) * R, :] = (
            res.results[core]["out"].astype(np.float32).reshape(O, R, W)
        )
    return out


# revision 15
# speedup vs baseline: 20.6474x; 20.2202x over previous
"""Depth-aware 3x3 convolution on 8 Trainium2 NeuronCores (Bass, raw engine blocks).

out[b,o,h,w] = sum_{c,kh,kw} weight[o,c,kh,kw] * x[b,c,h+kh-1,w+kw-1]
                             * exp(-8.3*|depth[b,h,w] - depth[b,h+kh-1,w+kw-1]|)

Sharding: core = 2*b + (h >= 128); each core computes a [32, 128, 256] output
slab from a 130-row padded input frame (1-row halo from the host slice).

Datapath is bf16 (x, weight, sim, modulated product, output) with f32 depth
and f32 PSUM accumulation; the DVE modulation multiply runs in 2x perf mode
(all operands contiguous, 4B-aligned, pitch-256 pre-shifted on the host).

DMA strategy: the HWDGE ring processes DMAs ~serially (~0.26us fixed each +
transfer at the dest-partition-port rate), so traffic is split across BOTH
rings and batched at row-pair granularity (fat descriptors):
  ring A (SP):  d/w loads, sim stores, x3 pair-chunk loads, out stores,
                sim broadcasts for pairs 6-7
  ring B (ACT): sim broadcasts for pairs 0-5, issued after the copy whose
                pe_sem wait already implies the needed DVE progress (no ACT
                self-waits - an ACT DMA racing ACT's own compute crashes).

Per-core pipeline (pair = 2 tiles = 16 rows = 8192 px):
  A. sim: depth rows pixel-major [128, 258]x3 -> sub (DVE) -> |.| (DVE STT)
     -> exp (ACT, bf16) -> DRAM simd[9, 32768]
  B. main loop over 16 tiles of 2048 px:
     - DMA: x3 pair chunk [96, 18*256] bf16 (3 column-shift blocks)
     - DMA: 9 per-tap pair broadcasts simd[k, 8192px] -> [32, 8192] bf16
     - DVE: xm3 = x3[...] * simrep3[...]  (bf16 2x)  t=0,1,2
     - PE : psum[32, 2048] += w3[:, t].T @ xm3  (K=96, N=512 x4, bf16)
     - ACT: psum -> out_sb bf16; SP stores out.

The body sits in a per-engine hardware loop (`trips`) with a two-phase
leader-follower barrier and exact semaphore reset between trips; device time
is measured as the wall-clock slope between two trip counts.  Grading uses
trips=1.
"""
import sys

import numpy as np

sys.path.insert(0, "/opt/trn_rl_repo")

import concourse.bass as bass
import concourse.mybir as mybir
from concourse.bass_utils import run_bass_kernel_spmd

F32 = mybir.dt.float32
BF16 = mybir.dt.bfloat16
EXP = mybir.ActivationFunctionType.Exp

B, C, H, W = 4, 32, 256, 256
O = 32
ALPHA = 8.3
R = 128  # output rows per core
WP = W + 2  # padded width (depth frame only)
FR = R + 2  # frame rows per core
NPIX = R * W  # 32768
TROWS = 8  # rows per tile
TILE = TROWS * W  # 2048
NT = R // TROWS  # 16
NP = NT // 2  # 8 pairs
PCH = (2 * TROWS + 2) * W  # x3 pair chunk elems per partition (18 rows)
PPX = 2 * TILE  # pixels per pair
PGT = 3 * PPX  # simrep pair group (3 passes x 4096)
MMN = 512  # matmul free-dim chunk
QN = TILE // MMN  # 4
XMB = 4  # xm ring depth

# bc ring/slot assignment: pairs 0-5 on ACT ring, 6-7 on SP ring
BC_RING = ["a", "a", "a", "a", "a", "a", "s", "s"]
# (sem index, cumulative wait value) per pair; sems: a0, a1, s0, s1
_BC_SEM = {}
_cum = {"a0": 0, "a1": 0, "s0": 0, "s1": 0}
for _p in range(NP):
    _key = BC_RING[_p] + str(_p % 2)
    _cum[_key] += 144
    _BC_SEM[_p] = (_key, _cum[_key])
BC_FINAL = dict(_cum)


def build_nc(trips=1):
    nc = bass.Bass("TRN2", target_bir_lowering=False, debug=False, num_devices=8)
    # x3: 3 column-shift blocks stacked on partitions, pitch-256 rows
    x3_in = nc.declare_dram_parameter("x3", [96, FR * W], BF16, isOutput=False)
    dp_in = nc.declare_dram_parameter("dp", [FR, WP], F32, isOutput=False)
    w3_in = nc.declare_dram_parameter("w3", [96, 96], BF16, isOutput=False)
    out_d = nc.declare_dram_parameter("out", [O, NPIX], BF16, isOutput=True)
    simd = nc.dram_tensor("simd", [9, NPIX], BF16)

    from contextlib import ExitStack

    ctx = ExitStack()
    with ctx:
        d_sb = ctx.enter_context(nc.sbuf_tensor([128, 3 * WP], F32))
        adiff9 = ctx.enter_context(nc.sbuf_tensor([128, 9 * W], F32))
        sim9 = ctx.enter_context(nc.sbuf_tensor([128, 9 * W], BF16))
        w3_sb = ctx.enter_context(nc.sbuf_tensor([96, 96], BF16))
        x3c = ctx.enter_context(nc.sbuf_tensor([96, 2 * PCH], BF16))
        simrep3 = ctx.enter_context(nc.sbuf_tensor([96, 2 * PGT], BF16))
        xm3 = ctx.enter_context(nc.sbuf_tensor([96, XMB * TILE], BF16))
        out_sb = ctx.enter_context(nc.sbuf_tensor([32, 2 * TILE], BF16))
        psum = ctx.enter_context(nc.psum_tensor([32, 2 * TILE], F32))
        ld_sem = ctx.enter_context(nc.semaphore("ld_sem"))
        sim_dve = ctx.enter_context(nc.semaphore("sim_dve"))
        act_exp = ctx.enter_context(nc.semaphore("act_exp"))
        sim_st = ctx.enter_context(nc.semaphore("sim_st"))
        x_q = [ctx.enter_context(nc.semaphore(f"x_q{r}")) for r in range(2)]
        bc_sems = {
            k: ctx.enter_context(nc.semaphore(f"bc_{k}"))
            for k in ("a0", "a1", "s0", "s1")
        }
        st_e = ctx.enter_context(nc.semaphore("st_e"))
        st_o = ctx.enter_context(nc.semaphore("st_o"))
        mod_sem = ctx.enter_context(nc.semaphore("mod_sem"))
        pe_sem = ctx.enter_context(nc.semaphore("pe_sem"))
        act_cp = ctx.enter_context(nc.semaphore("act_cp"))
        bar_g = ctx.enter_context(nc.semaphore("bar_g"))
        bar_r = ctx.enter_context(nc.semaphore("bar_r"))
        bar_a = ctx.enter_context(nc.semaphore("bar_a"))
        bar_r2 = ctx.enter_context(nc.semaphore("bar_r2"))
        block = ctx.enter_context(nc.Block())

        PIPE_SEMS = (
            [ld_sem, sim_dve, act_exp, sim_st, st_e, st_o, mod_sem, pe_sem, act_cp]
            + x_q + list(bc_sems.values())
        )

        def follower_barrier(eng):
            # two-phase: park on bar_r while SP resets pipe sems, then ack and
            # park on bar_r2 while SP resets bar_r.  All wait values are
            # trip-invariant; every sem returns to 0 each trip.
            eng.drain()
            eng.sem_inc(bar_g, 1)
            eng.wait_ge(bar_r, 1)
            eng.sem_inc(bar_a, 1)
            eng.wait_ge(bar_r2, 1)

        simd_r = simd.ap().rearrange("k (r w) -> k r w", r=R)

        def bc_pair(eng, p):
            # 9 per-tap broadcasts for pair p (2 tiles, 8192 px each)
            slot = p % 2
            sem = bc_sems[BC_RING[p] + str(slot)]
            for t in range(3):
                for j in range(3):
                    k = 3 * t + j
                    eng.dma_start(
                        simrep3[
                            32 * j : 32 * (j + 1),
                            slot * PGT + t * PPX : slot * PGT + (t + 1) * PPX,
                        ],
                        simd[k : k + 1, p * PPX : (p + 1) * PPX].to_broadcast(
                            (32, PPX)
                        ),
                    ).then_inc(sem, 16)

        @block.sync
        def _(sync: bass.BassEngine):
            with sync.Fori(0, trips):
                # startup loads: d (3 row-shifted views), w3
                for t in range(3):
                    sync.dma_start(
                        d_sb[:, t * WP : (t + 1) * WP], dp_in[t : t + 128, :]
                    ).then_inc(ld_sem, 16)
                sync.dma_start(w3_sb[:], w3_in[:]).then_inc(ld_sem, 16)
                # x3 pair chunks 0, 1 (buffers free at trip start)
                for p in range(2):
                    sync.dma_start(
                        x3c[:, (p % 2) * PCH : (p % 2 + 1) * PCH],
                        x3_in[:, p * 2 * TROWS * W : p * 2 * TROWS * W + PCH],
                    ).then_inc(x_q[p % 2], 16)
                # sim stores
                for k in range(9):
                    sync.wait_ge(act_exp, k + 1)
                    sync.dma_start(
                        simd_r[k], sim9[:, k * W : (k + 1) * W]
                    ).then_inc(sim_st, 16)
                # main loop: stores first (loosest waits), then SP-side
                # broadcasts, then x3 loads (tightest waits)
                for i in range(NT):
                    if i >= 1:
                        sync.wait_ge(act_cp, i)
                        sync.dma_start(
                            out_d[:, (i - 1) * TILE : i * TILE],
                            out_sb[:, ((i - 1) % 2) * TILE : ((i - 1) % 2 + 1) * TILE],
                        ).then_inc(st_e if (i - 1) % 2 == 0 else st_o, 16)
                    if i == 10:
                        sync.wait_ge(mod_sem, 30)
                        bc_pair(sync, 6)
                    if i == 12:
                        sync.wait_ge(mod_sem, 36)
                        bc_pair(sync, 7)
                    if i % 2 == 0:
                        p = i // 2 + 2
                        if p < NP:
                            sync.wait_ge(mod_sem, 6 * (p - 2) + 6)
                            sync.dma_start(
                                x3c[:, (p % 2) * PCH : (p % 2 + 1) * PCH],
                                x3_in[
                                    :, p * 2 * TROWS * W : p * 2 * TROWS * W + PCH
                                ],
                            ).then_inc(x_q[p % 2], 16)
                sync.wait_ge(act_cp, NT)
                sync.dma_start(
                    out_d[:, (NT - 1) * TILE :],
                    out_sb[:, ((NT - 1) % 2) * TILE : ((NT - 1) % 2 + 1) * TILE],
                ).then_inc(st_e if (NT - 1) % 2 == 0 else st_o, 16)
                # ---- trip barrier: leader ----
                sync.wait_ge(ld_sem, 64)
                sync.wait_ge(sim_st, 144)
                sync.wait_ge(x_q[0], 64)
                sync.wait_ge(x_q[1], 64)
                for k, v in BC_FINAL.items():
                    if v:
                        sync.wait_ge(bc_sems[k], v)
                sync.wait_ge(st_e, 128)
                sync.wait_ge(st_o, 128)
                sync.wait_ge(bar_g, 3)
                for sem in PIPE_SEMS:
                    sync.sem_clear(sem)
                sync.sem_clear(bar_g)
                sync.sem_clear(bar_r2)
                sync.sem_inc(bar_r, 1)
                sync.wait_ge(bar_a, 3)
                sync.sem_clear(bar_r)
                sync.sem_clear(bar_a)
                sync.sem_inc(bar_r2, 1)

        @block.vector
        def _(vector):
            with vector.Fori(0, trips):
                # sim phase: diff + abs per tap
                vector.wait_ge(ld_sem, 64)
                for t in range(3):
                    for j in range(3):
                        k = 3 * t + j
                        vector.tensor_sub(
                            adiff9[:, k * W : (k + 1) * W],
                            d_sb[:, WP + 1 : WP + 1 + W],
                            d_sb[:, t * WP + j : t * WP + j + W],
                        )
                        vector.drain()
                        vector.scalar_tensor_tensor(
                            adiff9[:, k * W : (k + 1) * W],
                            adiff9[:, k * W : (k + 1) * W],
                            -1.0,
                            adiff9[:, k * W : (k + 1) * W],
                            op0=mybir.AluOpType.mult,
                            op1=mybir.AluOpType.max,
                        ).then_inc(sim_dve, 1)
                # modulation loop
                for i in range(NT):
                    p = i // 2
                    slot = p % 2
                    vector.wait_ge(x_q[slot], 16 * (p // 2 + 1))
                    bk, bv = _BC_SEM[p]
                    vector.wait_ge(bc_sems[bk], bv)
                    for t in range(3):
                        s = 3 * i + t
                        sb = s % XMB
                        if s >= XMB:
                            vector.wait_ge(pe_sem, s - XMB + 1)
                        vector.tensor_mul(
                            xm3[:, sb * TILE : (sb + 1) * TILE],
                            x3c[
                                :,
                                slot * PCH
                                + ((i % 2) * TROWS + t) * W : slot * PCH
                                + ((i % 2) * TROWS + t) * W
                                + TILE,
                            ],
                            simrep3[
                                :,
                                slot * PGT
                                + t * PPX
                                + (i % 2) * TILE : slot * PGT
                                + t * PPX
                                + (i % 2) * TILE
                                + TILE,
                            ],
                        ).then_inc(mod_sem, 1)
                follower_barrier(vector)

        @block.tensor
        def _(tensor):
            with tensor.Fori(0, trips):
                tensor.wait_ge(ld_sem, 64)
                for i in range(NT):
                    pb = i % 2
                    if i >= 2:
                        tensor.wait_ge(act_cp, i - 1)
                    for t in range(3):
                        s = 3 * i + t
                        sb = s % XMB
                        tensor.wait_ge(mod_sem, s + 1)
                        for q in range(QN):
                            mm = tensor.matmul(
                                psum[
                                    :, pb * TILE + q * MMN : pb * TILE + (q + 1) * MMN
                                ],
                                w3_sb[:, 32 * t : 32 * (t + 1)],
                                xm3[:, sb * TILE + q * MMN : sb * TILE + (q + 1) * MMN],
                                start=(t == 0),
                                stop=(t == 2),
                            )
                            if q == QN - 1:
                                mm.then_inc(pe_sem, 1)
                follower_barrier(tensor)

        @block.scalar
        def _(scalar):
            with scalar.Fori(0, trips):
                # exp per tap (bf16 out)
                for k in range(9):
                    scalar.wait_ge(sim_dve, k + 1)
                    scalar.activation(
                        sim9[:, k * W : (k + 1) * W],
                        adiff9[:, k * W : (k + 1) * W],
                        EXP,
                        scale=-ALPHA,
                    ).then_inc(act_exp, 1)
                # ACT-ring broadcasts for pairs 0, 1 (after SP's sim stores)
                scalar.wait_ge(sim_st, 144)
                bc_pair(scalar, 0)
                bc_pair(scalar, 1)
                # psum -> sbuf copies; bc pairs 2-5 ride behind the copy whose
                # pe_sem wait implies the DVE progress they need
                for i in range(NT):
                    pb = i % 2
                    scalar.wait_ge(pe_sem, 3 * i + 3)
                    if i >= 2:
                        scalar.wait_ge(st_e if pb == 0 else st_o, 16 * (i // 2))
                    scalar.copy(
                        out_sb[:, pb * TILE : (pb + 1) * TILE],
                        psum[:, pb * TILE : (pb + 1) * TILE],
                    ).then_inc(act_cp, 1)
                    if i % 2 == 1 and 2 <= (i + 3) // 2 <= 5:
                        p = (i + 3) // 2
                        scalar.wait_ge(mod_sem, 6 * p - 6)
                        bc_pair(scalar, p)
                follower_barrier(scalar)

    return nc


_NC_CACHE = {}


def _get_nc(trips=1):
    if trips not in _NC_CACHE:
        _NC_CACHE[trips] = build_nc(trips)
    return _NC_CACHE[trips]


def _prep_core(x, depth, core):
    import ml_dtypes

    b, half = core // 2, core % 2
    r0 = half * R
    # padded frame [C, FR, WP]: image rows r0-1 .. r0+R, zero-padded
    xpad = np.zeros((C, FR, WP), dtype=np.float32)
    dpad = np.zeros((FR, WP), dtype=np.float32)
    lo, hi = r0 - 1, r0 + R + 1
    slo, shi = max(lo, 0), min(hi, H)
    xpad[:, slo - lo : shi - lo, 1 : 1 + W] = x[b, :, slo:shi, :]
    dpad[slo - lo : shi - lo, 1 : 1 + W] = depth[b, 0, slo:shi, :]
    # x3: 3 column-shift blocks stacked on partitions, pitch-256 (pre-shifted)
    x3 = np.empty((3, C, FR, W), dtype=np.float32)
    x3[0] = xpad[:, :, 0:W]  # j=0: w-1
    x3[1] = xpad[:, :, 1 : 1 + W]  # j=1: w
    x3[2] = xpad[:, :, 2 : 2 + W]  # j=2: w+1
    return {
        "x3": x3.reshape(3 * C, FR * W).astype(ml_dtypes.bfloat16),
        "dp": dpad,
        "w3": None,  # filled by caller (shared)
    }


def _prep_inputs(x, depth, weight):
    import ml_dtypes

    x = np.ascontiguousarray(x, dtype=np.float32)
    depth = np.ascontiguousarray(depth, dtype=np.float32)
    weight = np.ascontiguousarray(weight, dtype=np.float32)
    # w3[32j + c, 32t + o] = weight[o, c, t, j]
    w3 = (
        np.transpose(weight, (3, 1, 2, 0))
        .reshape(96, 96)
        .astype(ml_dtypes.bfloat16)
        .copy()
    )
    in_maps = []
    for core in range(8):
        m = _prep_core(x, depth, core)
        m["w3"] = w3
        in_maps.append(m)
    return in_maps


def kernel(x, depth, weight):
    in_maps = _prep_inputs(x, depth, weight)
    nc = _get_nc(1)
    res = run_bass_kernel_spmd(nc, in_maps, list(range(8)))

    out = np.empty((B, O, H, W), dtype=np.float32)
    for core in range(8):
        b, half = core // 2, core % 2
        out[b, :, half * R : (half + 1) * R, :] = (
            res.results[core]["out"].astype(np.float32).reshape(O, R, W)
        )
    return out
